# revision 4
# baseline (speedup 1.0000x reference)
"""Trainium2 Bass kernel for nn_BERT_936302870555.

Sharding: 8 cores; core c -> (stack = c//2, batch pair = (2*(c%2), 2*(c%2)+1)).
Every core runs the SAME 9-slot program with head schedule [1,2,3,4,6,9,12,18,36]
(stack0's schedule, a supersequence of every stack's schedule).  Shorter stacks
get identity slots (zero Wo/W2 -> residual passes through).  Activations are
feature-major [D=36 partitions, T=1200 tokens free].  A small second launch does
the SE-gated concat -> conv1d(k=3) -> BN -> ReLU epilogue, split 8 ways by
(batch, L-half).
"""

import numpy as np

D = 36
L = 600
B = 4
T = 2 * L          # tokens per core (2 batches)
NSLOT = 9
SCHED = [1, 2, 3, 4, 6, 9, 12, 18, 36]
HEADS = [1, 2, 3, 4, 6, 9, 12, 18, 36,
         1, 2, 3, 4, 6,
         6, 9, 12, 18, 36,
         3, 4, 6, 9, 12]
STACKS = [(0, 9), (9, 14), (14, 19), (19, 24)]
# slot offset of each stack's layers within SCHED
STACK_SLOT0 = {0: 0, 1: 0, 2: 4, 3: 2}
DFF = 144
LN_EPS = 1e-6
BN_EPS = 1e-5
NCHUNK = 3         # projection chunks over T
CW = T // NCHUNK   # 400
MCH = 5            # m-chunks of 120 per batch
MW = L // MCH      # 120
TH = 2             # t-halves of 300 per batch
TW = L // TH       # 300
CONV_W = 302       # conv input window per core (300 + halo)


def _group_layout(h):
    """For h heads of dk=36//h: list of (group_rows, [(strip, head_idx), ...]).
    dk<=32: pack up to 4 heads per group at 32-aligned strips; dk=36: single."""
    dk = D // h
    if dk > 32:
        return dk, [(36, [(0, 0)])]
    groups = []
    for g0 in range(0, h, 4):
        heads = list(range(g0, min(g0 + 4, h)))
        rows = 32 * len(heads)
        groups.append((rows, [(32 * j, i) for j, i in enumerate(heads)]))
    return dk, groups


def _sin_pe():
    pos = np.arange(L, dtype=np.float32)[:, None]
    i = np.arange(0, D, 2, dtype=np.float32)
    div = np.exp(-(np.log(10000.0) / D) * i)
    pe = np.zeros((L, D), dtype=np.float32)
    pe[:, 0::2] = np.sin(pos * div)
    pe[:, 1::2] = np.cos(pos * div)
    return pe.T.copy()  # [36, 600]


# ----- per-slot packed weight column layouts (shared by host packing + device) -----
def _qk_al_cols():
    offs, total = [], 0
    for h in SCHED:
        dk, groups = _group_layout(h)
        offs.append(total)
        total += sum(r for r, _ in groups) and 0 or 0  # placeholder
    return offs


_QKAL_OFF = []
_QKAL_TOT = 0
for _h in SCHED:
    _dk, _groups = _group_layout(_h)
    _QKAL_OFF.append(_QKAL_TOT)
    _QKAL_TOT += sum(_r for _r, _ in _groups)

_VIL_OFF = []
_VIL_TOT = 0
for _h in SCHED:
    _VIL_OFF.append(_VIL_TOT)
    _VIL_TOT += D + _h

_GG_OFF = []
_GG_TOT = 0
for _h in SCHED:
    _GG_OFF.append(_GG_TOT)
    _GG_TOT += len(_group_layout(_h)[1])


def _split_multi_waits(nc):
    """This container's walrus encodes at most ONE sem-wait per instruction.
    Tile emits multi-wait instructions; split the extras into single-wait
    EventSemaphore nops inserted immediately before, on the same engine."""
    import concourse.mybir as mybir

    n = 0
    for f in nc.m.functions:
        for bb in f.blocks:
            il = bb.instructions
            i = 0
            while i < len(il):
                inst = il[i]
                si = inst.sync_info
                if si is not None and si.on_wait and len(si.on_wait) > 1:
                    waits = list(si.on_wait)
                    for w in waits[:-1]:
                        ev = mybir.InstEventSemaphore(
                            name=f"I-wsplit-{n}",
                            engine=inst.engine,
                            ins=[], outs=[],
                            sync_info=mybir.SyncInfo(on_wait=[w], on_update=[]),
                        )
                        n += 1
                        il.insert(i, ev)
                        i += 1
                    inst.sync_info = mybir.SyncInfo(on_wait=[waits[-1]],
                                                    on_update=list(si.on_update or []))
                i += 1
    return n


def _build_nc():
    import concourse.bass as bass
    import concourse.mybir as mybir
    from concourse.tile import TileContext

    f32 = mybir.dt.float32
    AF = mybir.ActivationFunctionType
    OP = mybir.AluOpType

    nc = bass.Bass()
    dp = lambda name, shape, dtype, isOutput=False: nc.declare_dram_parameter(name, shape, dtype, isOutput)
    xin_d = dp("xin", [D, T], f32)
    ident_d = dp("ident", [128, 128], f32)
    lng_d = dp("lng", [D, 2 * NSLOT], f32)     # ln1_g | ln2_g per slot (cols 2s, 2s+1)
    lnb_d = dp("lnb", [D, 2 * NSLOT], f32)
    wqal_d = dp("wqal", [D, _QKAL_TOT], f32)   # padded/aligned q-group weights (x 1/sqrt(dk))
    wkal_d = dp("wkal", [D, _QKAL_TOT], f32)
    bqal_d = dp("bqal", [128, NSLOT], f32)     # aligned q bias per slot (padded rows)
    bkal_d = dp("bkal", [128, NSLOT], f32)
    wvil_d = dp("wvil", [D, _VIL_TOT], f32)    # interleaved V weights (+zero ones-cols)
    bvil_d = dp("bvil", [D + SCHED[-1], NSLOT], f32)  # interleaved v bias (+1.0 at ones rows)
    woal_d = dp("wo_al", [128, _GG_TOT * D], f32)
    exal_d = dp("exal", [4, _GG_TOT * 128], f32)
    bo_d = dp("bo", [D, NSLOT], f32)
    w1_d = dp("w1", [D, NSLOT * DFF], f32)
    b1_d = dp("b1", [72, 2 * NSLOT], f32)      # col 2s+half
    w2_d = dp("w2", [72, NSLOT * 2 * D], f32)  # slot s half q: cols (2s+q)*36
    b2_d = dp("b2", [D, NSLOT], f32)
    out_d = dp("zout", [D, T], f32, isOutput=True)

    VILW = D + SCHED[-1]  # 72 max interleaved width

    with TileContext(nc) as tc:
        with tc.tile_pool(name="const", bufs=1) as cpool, \
             tc.tile_pool(name="state", bufs=1) as spool, \
             tc.tile_pool(name="work", bufs=2) as wpool, \
             tc.tile_pool(name="attn", bufs=1) as apool, \
             tc.tile_pool(name="ffp", bufs=1) as ffpool, \
             tc.tile_pool(name="ps_proj", bufs=2, space="PSUM") as pp, \
             tc.tile_pool(name="ps_stat", bufs=2, space="PSUM") as pstat, \
             tc.tile_pool(name="ps_attn", bufs=2, space="PSUM") as pa, \
             tc.tile_pool(name="ps_oa", bufs=2, space="PSUM") as poap:

            def load(dram, shape):
                t = cpool.tile(shape, f32, tag=dram.name)
                nc.sync.dma_start(out=t[:], in_=dram[:])
                return t

            ident = load(ident_d, [128, 128])
            ones36 = cpool.tile([D, 1], f32, tag="ones36")
            nc.vector.memset(ones36[:], 1.0)
            ones1x36 = cpool.tile([1, D], f32, tag="ones1x36")
            nc.vector.memset(ones1x36[:], 1.0)
            lng = load(lng_d, [D, 2 * NSLOT]); lnb = load(lnb_d, [D, 2 * NSLOT])
            wqal = load(wqal_d, [D, _QKAL_TOT]); wkal = load(wkal_d, [D, _QKAL_TOT])
            bqal = load(bqal_d, [128, NSLOT]); bkal = load(bkal_d, [128, NSLOT])
            wvil = load(wvil_d, [D, _VIL_TOT]); bvil = load(bvil_d, [VILW, NSLOT])
            wo_al = load(woal_d, [128, _GG_TOT * D])
            exal = load(exal_d, [4, _GG_TOT * 128])
            bo = load(bo_d, [D, NSLOT])
            w1 = load(w1_d, [D, NSLOT * DFF]); b1 = load(b1_d, [72, 2 * NSLOT])
            w2 = load(w2_d, [72, NSLOT * 2 * D]); b2 = load(b2_d, [D, NSLOT])

            z = spool.tile([D, T], f32, tag="z")
            nc.sync.dma_start(out=z[:], in_=xin_d[:])

            def layernorm(src, gcol, bcol, ydst):
                zsq = ffpool.tile([D, T], f32, tag="zsq")
                nc.scalar.activation(out=zsq[:], in_=src[:], func=AF.Square)
                # stats blocks along free dim of one partition-0 row:
                # [0:T]=mu, [T:2T]=ex2->var->std, [2T:3T]=mu^2 then rstd
                st = ffpool.tile([1, 3 * T], f32, tag="lnstats")
                mu, va, sq = st[0:1, 0:T], st[0:1, T:2 * T], st[0:1, 2 * T:3 * T]
                for c in range(NCHUNK):
                    cs = slice(c * CW, (c + 1) * CW)
                    ps_s = pstat.tile([1, CW], f32, tag="lnsum")
                    ps_q = pstat.tile([1, CW], f32, tag="lnsum")
                    nc.tensor.matmul(out=ps_s[:], lhsT=ones36[:], rhs=src[:, cs])
                    nc.tensor.matmul(out=ps_q[:], lhsT=ones36[:], rhs=zsq[:, cs])
                    nc.vector.tensor_scalar(out=st[0:1, c * CW:(c + 1) * CW],
                                            in0=ps_s[:], scalar1=1.0 / D,
                                            scalar2=None, op0=OP.mult)
                    nc.vector.tensor_scalar(out=st[0:1, T + c * CW:T + (c + 1) * CW],
                                            in0=ps_q[:], scalar1=1.0 / D,
                                            scalar2=None, op0=OP.mult)
                nc.vector.tensor_tensor(out=sq, in0=mu, in1=mu, op=OP.mult)
                nc.vector.tensor_tensor(out=va, in0=va, in1=sq, op=OP.subtract)
                nc.vector.tensor_scalar(out=va, in0=va, scalar1=LN_EPS,
                                        scalar2=None, op0=OP.add)
                nc.scalar.activation(out=va, in_=va, func=AF.Sqrt)
                rstd = sq
                nc.vector.reciprocal(out=rstd, in_=va)
                # broadcast mu/rstd across partitions via K=1 ones-matmuls
                t0 = ffpool.tile([D, T], f32, tag="lnt0")
                for c in range(NCHUNK):
                    cs = slice(c * CW, (c + 1) * CW)
                    mu_b = pstat.tile([D, CW], f32, tag="lnsum", name="mu_b")
                    rs_b = pstat.tile([D, CW], f32, tag="lnsum", name="rs_b")
                    nc.tensor.matmul(out=mu_b[:], lhsT=ones1x36[:],
                                     rhs=st[0:1, c * CW:(c + 1) * CW])
                    nc.tensor.matmul(out=rs_b[:], lhsT=ones1x36[:],
                                     rhs=st[0:1, 2 * T + c * CW:2 * T + (c + 1) * CW])
                    nc.vector.tensor_tensor(out=t0[:, cs], in0=src[:, cs],
                                            in1=mu_b[:], op=OP.subtract)
                    nc.vector.tensor_tensor(out=t0[:, cs], in0=t0[:, cs],
                                            in1=rs_b[:], op=OP.mult)
                nc.scalar.activation(out=ydst[:], in_=t0[:], func=AF.Identity,
                                     bias=bcol, scale=gcol)

            for s in range(NSLOT):
                h = SCHED[s]
                dk, groups = _group_layout(h)
                scale = 1.0  # 1/sqrt(dk) folded into wqal on host
                y = wpool.tile([D, T], f32, tag="y")
                layernorm(z, lng[:, 2 * s:2 * s + 1], lnb[:, 2 * s:2 * s + 1], y)

                # --- V interleaved projection ---
                vw = D + h
                vai = apool.tile([VILW, T], f32, tag="vai")
                vcol = _VIL_OFF[s]
                for c in range(NCHUNK):
                    psv = pp.tile([128, CW], f32, tag="psq")
                    nc.tensor.matmul(out=psv[0:vw, :],
                                     lhsT=wvil[:, vcol:vcol + vw],
                                     rhs=y[:, c * CW:(c + 1) * CW])
                    nc.scalar.activation(out=vai[0:vw, c * CW:(c + 1) * CW],
                                         in_=psv[0:vw, :], func=AF.Identity,
                                         bias=bvil[0:vw, s:s + 1])

                oacc = apool.tile([D, T], f32, tag="oacc")
                nc.vector.memset(oacc[:], 0.0)
                vtis = []
                for b in range(2):
                    toff = b * L
                    # transpose [V;1]-interleaved -> vti [120, 5*vw]
                    vti = apool.tile([MW, MCH * VILW], f32, tag=f"vti{b}")
                    for c in range(MCH):
                        pvt = pa.tile([MW, TW], f32, tag="pst")
                        nc.tensor.transpose(pvt[:, 0:vw],
                                            vai[0:vw, toff + c * MW: toff + (c + 1) * MW],
                                            ident[0:vw, 0:vw])
                        nc.vector.tensor_copy(out=vti[:, c * VILW:c * VILW + vw],
                                              in_=pvt[:, 0:vw])
                    vtis.append(vti)

                col = _QKAL_OFF[s]
                for gl, (rows, strips) in enumerate(groups):
                    gg = _GG_OFF[s] + gl
                    nh = len(strips)
                    gh = rows + 1 if h == 1 else rows
                    # Q/K aligned projection for this head group
                    qg = apool.tile([128, T], f32, tag="qal")
                    kg = apool.tile([128, T], f32, tag="kal")
                    for c in range(NCHUNK):
                        psq = pp.tile([128, CW], f32, tag="psq")
                        psk = pp.tile([128, CW], f32, tag="psq")
                        nc.tensor.matmul(out=psq[0:rows, :],
                                         lhsT=wqal[:, col:col + rows],
                                         rhs=y[:, c * CW:(c + 1) * CW])
                        nc.tensor.matmul(out=psk[0:rows, :],
                                         lhsT=wkal[:, col:col + rows],
                                         rhs=y[:, c * CW:(c + 1) * CW])
                        nc.scalar.activation(out=qg[0:rows, c * CW:(c + 1) * CW],
                                             in_=psq[0:rows, :], func=AF.Identity,
                                             bias=bqal[0:rows, s:s + 1])
                        nc.scalar.activation(out=kg[0:rows, c * CW:(c + 1) * CW],
                                             in_=psk[0:rows, :], func=AF.Identity,
                                             bias=bkal[0:rows, s:s + 1])
                    col += rows
                    krows = 32 if dk <= 32 else 36
                    onorm = apool.tile([128, T], f32, tag="onorm")
                    for b in range(2):
                        toff = b * L
                        vti = vtis[b]
                        # scores + exp, per head in group, m-chunk, t-half
                        etiles = {}
                        for strip, i in strips:
                            etiles[i] = apool.tile([MW, MCH * L], f32,
                                                   tag=f"e{strip}", name=f"e{strip}")
                        for c in range(MCH):
                            for th in range(TH):
                                for strip, i in strips:
                                    pst = pa.tile([MW, TW], f32, tag="pst")
                                    nc.tensor.matmul(
                                        out=pst[:],
                                        lhsT=kg[strip:strip + krows,
                                                toff + c * MW: toff + (c + 1) * MW],
                                        rhs=qg[strip:strip + krows,
                                               toff + th * TW: toff + (th + 1) * TW],
                                        tile_position=(strip, 0))
                                    nc.scalar.activation(
                                        out=etiles[i][:, c * L + th * TW:
                                                      c * L + (th + 1) * TW],
                                        in_=pst[:], func=AF.Exp, scale=scale)
                        # AV + normalizer rows; col-tiled into one psum tile.
                        # vti head blocks are [ones | V_i^T], so row strip+0 = Z_i
                        # and rows strip+1..strip+dk = O'_i.
                        zgrz = apool.tile([8, L], f32, tag="zg", name="zgrz")
                        zg = zgrz[0:4, :]
                        po_th = []
                        for th in range(TH):
                            poa = poap.tile([128, TW], f32, tag="poa")
                            po_th.append(poa)
                            for strip, i in strips:
                                obase = strip if dk <= 31 else 0
                                for c in range(MCH):
                                    nc.tensor.matmul(
                                        out=poa[obase:obase + dk + 1, :],
                                        lhsT=vti[:, c * VILW + i * (dk + 1):
                                                 c * VILW + (i + 1) * (dk + 1)],
                                        rhs=etiles[i][:, c * L + th * TW:
                                                      c * L + (th + 1) * TW],
                                        start=(c == 0), stop=(c == MCH - 1),
                                        tile_position=(0, obase))
                            osl = onorm[0:gh, toff + th * TW: toff + (th + 1) * TW]
                            nc.vector.tensor_copy(out=osl, in_=poa[0:gh, :])
                            # Z rows sit at strip bases {0,32,..} of onorm; DMA-gather
                            # them (engines can't do strided-partition access)
                            if dk <= 31:
                                nc.sync.dma_start(
                                    out=zg[0:nh, th * TW:(th + 1) * TW],
                                    in_=onorm[0:32 * nh:32,
                                              toff + th * TW: toff + (th + 1) * TW])
                            else:
                                nc.vector.tensor_copy(
                                    out=zg[0:1, th * TW:(th + 1) * TW],
                                    in_=poa[0:1, :])
                        rz = apool.tile([4, L], f32, tag="rz", name="rz")
                        nc.vector.reciprocal(out=rz[0:nh, :], in_=zg[0:nh, :])
                        for th in range(TH):
                            pd = pstat.tile([128, TW], f32, tag="lnsum", name="pd")
                            nc.tensor.matmul(
                                out=pd[0:gh, :],
                                lhsT=exal[0:nh, gg * 128: gg * 128 + gh],
                                rhs=rz[0:nh, th * TW:(th + 1) * TW])
                            osl = onorm[0:gh, toff + th * TW: toff + (th + 1) * TW]
                            nc.vector.tensor_tensor(out=osl, in0=osl,
                                                    in1=pd[0:gh, :], op=OP.mult)
                    # O-projection accumulate for this head group
                    for c in range(NCHUNK):
                        po = pp.tile([128, CW], f32, tag="psq", name="po")
                        nc.tensor.matmul(out=po[0:D, :],
                                         lhsT=wo_al[0:gh, gg * D:(gg + 1) * D],
                                         rhs=onorm[0:gh, c * CW:(c + 1) * CW])
                        nc.vector.tensor_tensor(out=oacc[:, c * CW:(c + 1) * CW],
                                                in0=oacc[:, c * CW:(c + 1) * CW],
                                                in1=po[0:D, :], op=OP.add)

                # --- attention bias + residual ---
                ob = ffpool.tile([D, T], f32, tag="ob")
                nc.vector.tensor_scalar(out=ob[:], in0=oacc[:], scalar1=bo[:, s:s + 1],
                                        scalar2=None, op0=OP.add)
                nc.vector.tensor_tensor(out=z[:], in0=z[:], in1=ob[:], op=OP.add)

                # --- FFN ---
                y2 = wpool.tile([D, T], f32, tag="y")
                layernorm(z, lng[:, 2 * s + 1:2 * s + 2], lnb[:, 2 * s + 1:2 * s + 2], y2)
                ff = ffpool.tile([72, 2 * T], f32, tag="ff")  # halves side by side
                for half in range(2):
                    for c in range(NCHUNK):
                        psf = pp.tile([128, CW], f32, tag="psq")
                        nc.tensor.matmul(
                            out=psf[0:72, :],
                            lhsT=w1[:, s * DFF + half * 72: s * DFF + (half + 1) * 72],
                            rhs=y2[:, c * CW:(c + 1) * CW])
                        nc.scalar.activation(
                            out=ff[:, half * T + c * CW: half * T + (c + 1) * CW],
                            in_=psf[0:72, :], func=AF.Gelu_apprx_tanh,
                            bias=b1[:, 2 * s + half:2 * s + half + 1])
                for c in range(NCHUNK):
                    psf2 = pp.tile([128, CW], f32, tag="psq")
                    for half in range(2):
                        nc.tensor.matmul(
                            out=psf2[0:D, :],
                            lhsT=w2[:, (2 * s + half) * D:(2 * s + half + 1) * D],
                            rhs=ff[:, half * T + c * CW: half * T + (c + 1) * CW],
                            start=(half == 0), stop=(half == 1))
                    fb = wpool.tile([D, CW], f32, tag="ob")
                    nc.scalar.activation(out=fb[:], in_=psf2[0:D, :], func=AF.Identity,
                                         bias=b2[:, s:s + 1])
                    nc.vector.tensor_tensor(out=z[:, c * CW:(c + 1) * CW],
                                            in0=z[:, c * CW:(c + 1) * CW],
                                            in1=fb[:], op=OP.add)

            # --- SE gating per batch, write out ---
            gated = spool.tile([D, T], f32, tag="gated")
            gsc = wpool.tile([D, 4], f32, tag="gsc")
            for b in range(2):
                toff = b * L
                nc.vector.tensor_reduce(out=gsc[:, b:b + 1], in_=z[:, toff:toff + L],
                                        axis=mybir.AxisListType.X, op=OP.add)
                nc.scalar.activation(out=gsc[:, 2 + b:3 + b], in_=gsc[:, b:b + 1],
                                     func=AF.Sigmoid, scale=1.0 / L)
                nc.vector.tensor_scalar(out=gated[:, toff:toff + L],
                                        in0=z[:, toff:toff + L],
                                        scalar1=gsc[:, 2 + b:3 + b],
                                        scalar2=None, op0=OP.mult)
            nc.sync.dma_start(out=out_d[:], in_=gated[:])
    return nc


def _build_nc2():
    """Launch 2: conv1d(144->36,k=3,pad=1) + BN + ReLU on a [144, 302] window.
    Channel dim split into two 72-row halves; all inputs merged into one
    tensor per consumer so no instruction waits on two DMAs."""
    import concourse.bass as bass
    import concourse.mybir as mybir
    from concourse.tile import TileContext

    f32 = mybir.dt.float32
    AF = mybir.ActivationFunctionType
    XW = 2 * CONV_W
    WW = 2 * 3 * D

    nc = bass.Bass()
    xfw_d = nc.declare_dram_parameter("xfw", [72, XW + WW], f32, False)
    sb_d = nc.declare_dram_parameter("sb2", [D, 2], f32, False)
    out_d = nc.declare_dram_parameter("yout", [D, TW], f32, True)

    with TileContext(nc) as tc:
        with tc.tile_pool(name="sb", bufs=1) as sb, \
             tc.tile_pool(name="ps", bufs=2, space="PSUM") as ps:
            xfw = sb.tile([72, XW + WW], f32, tag="xfw")
            nc.sync.dma_start(out=xfw[:], in_=xfw_d[:])
            sb2 = sb.tile([D, 2], f32, tag="sb2")
            nc.sync.dma_start(out=sb2[:], in_=sb_d[:])

            pso = ps.tile([D, TW], f32, tag="pso")
            first = True
            for k in range(3):
                for half in range(2):
                    nc.tensor.matmul(
                        out=pso[:],
                        lhsT=xfw[:, XW + half * 3 * D + k * D:
                                 XW + half * 3 * D + (k + 1) * D],
                        rhs=xfw[:, half * CONV_W + k: half * CONV_W + k + TW],
                        start=first, stop=(k == 2 and half == 1))
                    first = False
            yo = sb.tile([D, TW], f32, tag="yo")
            nc.scalar.activation(out=yo[:], in_=pso[:], func=AF.Relu,
                                 scale=sb2[:, 0:1], bias=sb2[:, 1:2])
            nc.sync.dma_start(out=out_d[:], in_=yo[:])
    return nc


_CACHE = {}
LAST_RESULTS = []  # per-launch BassKernelResults from the most recent kernel() call


def _pack_core_weights(stack, Wq, bq, Wk, bk, Wv, bv, Wo, bo,
                       ln1_g, ln1_b, ln2_g, ln2_b, W1, b1, W2, b2):
    """Build per-core packed weight arrays with identity padding."""
    s0 = STACK_SLOT0[stack]
    lo, hi = STACKS[stack]
    nlay = hi - lo

    lng = np.zeros((D, 2 * NSLOT), np.float32)
    lnb = np.zeros((D, 2 * NSLOT), np.float32)
    lng[:, :] = 1.0
    wqal = np.zeros((D, _QKAL_TOT), np.float32)
    wkal = np.zeros((D, _QKAL_TOT), np.float32)
    bqal = np.zeros((128, NSLOT), np.float32)
    bkal = np.zeros((128, NSLOT), np.float32)
    wvil = np.zeros((D, _VIL_TOT), np.float32)
    bvil = np.zeros((D + SCHED[-1], NSLOT), np.float32)
    wo_al = np.zeros((128, _GG_TOT * D), np.float32)
    exal = np.zeros((4, _GG_TOT * 128), np.float32)
    bo_p = np.zeros((D, NSLOT), np.float32)
    w1_p = np.zeros((D, NSLOT * DFF), np.float32)
    b1_p = np.zeros((72, 2 * NSLOT), np.float32)
    w2_p = np.zeros((72, NSLOT * 2 * D), np.float32)
    b2_p = np.zeros((D, NSLOT), np.float32)

    for s in range(NSLOT):
        h = SCHED[s]
        dk, groups = _group_layout(h)
        li = lo + (s - s0)
        real = s0 <= s < s0 + nlay
        # ones-rows of the interleaved V bias are always 1.0 (block-FIRST position)
        for i in range(h):
            bvil[i * (dk + 1), s] = 1.0
        # exal: expand normalizer rows back to the aligned AV output layout
        for gl in range(len(groups)):
            gg = _GG_OFF[s] + gl
            nh = len(groups[gl][1])
            for j in range(nh):
                base = (1 if h == 1 else 32 * j + 1)
                exal[j, gg * 128 + base: gg * 128 + base + dk] = 1.0
        if not real:
            continue
        lng[:, 2 * s] = ln1_g[li]; lnb[:, 2 * s] = ln1_b[li]
        lng[:, 2 * s + 1] = ln2_g[li]; lnb[:, 2 * s + 1] = ln2_b[li]
        sc = 1.0 / np.sqrt(dk)
        wq_s = Wq[li] * sc
        bq_s = bq[li] * sc
        col = _QKAL_OFF[s]
        for rows, strips in groups:
            for strip, i in strips:
                wqal[:, col + strip: col + strip + dk] = wq_s[:, i * dk:(i + 1) * dk]
                wkal[:, col + strip: col + strip + dk] = Wk[li][:, i * dk:(i + 1) * dk]
                bqal[strip:strip + dk, s] = bq_s[i * dk:(i + 1) * dk]
                bkal[strip:strip + dk, s] = bk[li][i * dk:(i + 1) * dk]
            col += rows
        vcol = _VIL_OFF[s]
        for i in range(h):
            wvil[:, vcol + i * (dk + 1) + 1: vcol + (i + 1) * (dk + 1)] = \
                Wv[li][:, i * dk:(i + 1) * dk]
            bvil[i * (dk + 1) + 1:(i + 1) * (dk + 1), s] = bv[li][i * dk:(i + 1) * dk]
        for gl, (rows, strips) in enumerate(groups):
            gg = _GG_OFF[s] + gl
            for j, i in strips:
                base = (1 if h == 1 else j + 1)
                wo_al[base:base + dk, gg * D:(gg + 1) * D] = \
                    Wo[li][i * dk:(i + 1) * dk, :]
        bo_p[:, s] = bo[li]
        w1_p[:, s * DFF:(s + 1) * DFF] = W1[li]
        b1_p[:, 2 * s] = b1[li][:72]; b1_p[:, 2 * s + 1] = b1[li][72:]
        w2_p[:, 2 * s * D:(2 * s + 1) * D] = W2[li][:72]
        w2_p[:, (2 * s + 1) * D:(2 * s + 2) * D] = W2[li][72:]
        b2_p[:, s] = b2[li]
    return dict(lng=lng, lnb=lnb, wqal=wqal, wkal=wkal, bqal=bqal, bkal=bkal,
                wvil=wvil, bvil=bvil, wo_al=wo_al, exal=exal, bo=bo_p,
                w1=w1_p, b1=b1_p, w2=w2_p, b2=b2_p)


def kernel(x, ln1_g, ln1_b, Wq, bq, Wk, bk, Wv, bv, Wo, bo,
           ln2_g, ln2_b, W1, b1, W2, b2,
           conv_w, conv_b, bn_g, bn_b, bn_mean, bn_var):
    from concourse.bass_utils import run_bass_kernel_spmd

    args = [np.asarray(a, np.float32) for a in
            (x, ln1_g, ln1_b, Wq, bq, Wk, bk, Wv, bv, Wo, bo,
             ln2_g, ln2_b, W1, b1, W2, b2)]
    (x, ln1_g, ln1_b, Wq, bq, Wk, bk, Wv, bv, Wo, bo,
     ln2_g, ln2_b, W1, b1, W2, b2) = args
    conv_w = np.asarray(conv_w, np.float32)
    conv_b = np.asarray(conv_b, np.float32)
    bn_g = np.asarray(bn_g, np.float32); bn_b = np.asarray(bn_b, np.float32)
    bn_mean = np.asarray(bn_mean, np.float32); bn_var = np.asarray(bn_var, np.float32)

    if "nc1" not in _CACHE:
        _CACHE["nc1"] = _build_nc()
        _split_multi_waits(_CACHE["nc1"])
        _CACHE["nc2"] = _build_nc2()
        _split_multi_waits(_CACHE["nc2"])
    nc1, nc2 = _CACHE["nc1"], _CACHE["nc2"]

    pe = _sin_pe()
    ident = np.eye(128, dtype=np.float32)

    in_maps = []
    for c in range(8):
        stack, bp = c // 2, c % 2
        b0, b1i = 2 * bp, 2 * bp + 1
        packed = _pack_core_weights(stack, Wq, bq, Wk, bk, Wv, bv, Wo, bo,
                                    ln1_g, ln1_b, ln2_g, ln2_b, W1, b1, W2, b2)
        m = dict(xin=(np.concatenate([x[b0], x[b1i]], axis=1)
                      + np.tile(pe, (1, 2))).astype(np.float32),
                 ident=ident, **packed)
        in_maps.append(m)
    LAST_RESULTS.clear()
    r1 = run_bass_kernel_spmd(nc1, in_maps, list(range(8)))
    LAST_RESULTS.append(r1)
    res1 = r1.results

    # gated branch outputs: res1[c]["zout"] = [36, 1200] for (stack c//2, batches)
    # assemble xf [B, 144, 600]
    xf = np.zeros((B, DFF, L), np.float32)
    for c in range(8):
        stack, bp = c // 2, c % 2
        zo = res1[c]["zout"]
        xf[2 * bp, stack * D:(stack + 1) * D] = zo[:, :L]
        xf[2 * bp + 1, stack * D:(stack + 1) * D] = zo[:, L:]

    scale = bn_g / np.sqrt(bn_var + BN_EPS)
    bias = bn_b + scale * (conv_b - bn_mean)
    wc = np.zeros((DFF, 3 * D), np.float32)
    for k in range(3):
        wc[:, k * D:(k + 1) * D] = conv_w[:, :, k].T
    wc2 = wc.reshape(2, 72, 3 * D).transpose(1, 0, 2).reshape(72, 2 * 3 * D).copy()
    in_maps2 = []
    for c in range(8):
        b, half = c // 2, c % 2
        win = np.zeros((DFF, CONV_W), np.float32)
        lo = half * TW - 1
        s0 = max(lo, 0)
        s1 = min(lo + CONV_W, L)
        win[:, s0 - lo: s1 - lo] = xf[b][:, s0:s1]
        win2 = win.reshape(2, 72, CONV_W).transpose(1, 0, 2).reshape(72, 2 * CONV_W)
        xfw = np.concatenate([win2, wc2], axis=1).astype(np.float32)
        sb2 = np.stack([scale, bias], axis=1).astype(np.float32)
        in_maps2.append(dict(xfw=xfw.copy(), sb2=sb2.copy()))
    r2 = run_bass_kernel_spmd(nc2, in_maps2, list(range(8)))
    LAST_RESULTS.append(r2)
    res2 = r2.results

    out = np.zeros((B, D, L), np.float32)
    for c in range(8):
        b, half = c // 2, c % 2
        out[b][:, half * TW:(half + 1) * TW] = res2[c]["yout"]
    return out



# revision 27
# speedup vs baseline: 1.4346x; 1.4346x over previous
"""Trainium2 Bass kernel for nn_BERT_936302870555 (v2).

Sharding: 8 cores; core c -> (batch b = c%4, group g = c//4). Each core's
1200-token row is two 600-token halves at different layer chains:
  g=0: half0 = stack0 chain (9 layers), half1 = s1 chain (5 layers)
  g=1: half0 = s2 chain (5 layers),    half1 = s3 chain (5 layers)
Program schedule (same for all cores): half0 heads H0=[1,2,3,4,6,9,12,18,36],
half1 heads H1=[1,2,3,4,6,9,12]; chains embed as subsequences, missing
(slot,half) entries run with zero weights (residual pass-through).
All matmuls run as float32r (1 cyc/row at free>=256); fp32 everywhere else.
Second launch: SE-gated concat -> conv1d(k=3) -> BN -> ReLU epilogue.
"""

import numpy as np

D = 36
L = 600
B = 4
T = 2 * L
NSLOT = 9
H0 = [1, 2, 3, 4, 6, 9, 12, 18, 36]
H1 = [1, 2, 3, 4, 6, 9, 12]
HEADS = [1, 2, 3, 4, 6, 9, 12, 18, 36,
         1, 2, 3, 4, 6,
         6, 9, 12, 18, 36,
         3, 4, 6, 9, 12]
DFF = 144
LN_EPS = 1e-6
BN_EPS = 1e-5
CH = 300           # token chunk within a half
MW = 120           # key-chunk rows
TH = 2             # 300-token q-chunks per half
VSTR = 72          # fixed per-m-chunk stride in vti
CONV_W = 302
TW = 300

# (slot, half) -> heads; program-level, core-independent
SH = []
for _s in range(NSLOT):
    SH.append((_s, 0, H0[_s]))
    if _s < len(H1):
        SH.append((_s, 1, H1[_s]))
ESH = {(s, j): e for e, (s, j, _h) in enumerate(SH)}
NSH = len(SH)  # 16


def _group_layout(h):
    """For h heads of dk=36//h: (dk, [(rows, [(strip, head_idx), ...]), ...])."""
    dk = D // h
    if dk > 32:
        return dk, [(36, [(0, 0)])]
    groups = []
    for g0 in range(0, h, 4):
        heads = list(range(g0, min(g0 + 4, h)))
        groups.append((32 * len(heads), [(32 * j, i) for j, i in enumerate(heads)]))
    return dk, groups


_QKOFF = {}
_QKTOT = 0
_VOFF = {}
_VTOT = 0
_GGOFF = {}
_GGTOT = 0
_EXKEY = {}
for _s, _j, _h in SH:
    _dk, _groups = _group_layout(_h)
    _QKOFF[(_s, _j)] = _QKTOT
    _QKTOT += sum(r for r, _ in _groups)
    _VOFF[(_s, _j)] = _VTOT
    _VTOT += D + _h + ((D + _h) % 2)
    _GGOFF[(_s, _j)] = _GGTOT
    _GGTOT += len(_groups)
    for _rows, _strips in _groups:
        _k = (_dk, len(_strips))
        if _k not in _EXKEY:
            _EXKEY[_k] = len(_EXKEY)
_NEXK = len(_EXKEY)


def _sin_pe():
    pos = np.arange(L, dtype=np.float32)[:, None]
    i = np.arange(0, D, 2, dtype=np.float32)
    div = np.exp(-(np.log(10000.0) / D) * i)
    pe = np.zeros((L, D), dtype=np.float32)
    pe[:, 0::2] = np.sin(pos * div)
    pe[:, 1::2] = np.cos(pos * div)
    return pe.T.copy()  # [36, 600]


def _chain_layer(grp, s, j):
    """Layer index for (slot, half) on cores of group grp, or None."""
    if grp == 0:
        if j == 0:
            return s                      # stack0: layers 0..8
        return 9 + s if s < 5 else None   # s1: layers 9..13 at slots 0..4
    else:
        if j == 0:
            return 14 + (s - 4) if s >= 4 else None  # s2 at slots 4..8
        return 19 + (s - 2) if 2 <= s < 7 else None  # s3 at slots 2..6


def _split_multi_waits(nc):
    """This container's walrus encodes at most ONE sem-wait per instruction."""
    import concourse.mybir as mybir

    n = 0
    for f in nc.m.functions:
        for bb in f.blocks:
            il = bb.instructions
            i = 0
            while i < len(il):
                inst = il[i]
                si = inst.sync_info
                if si is not None and si.on_wait and len(si.on_wait) > 1:
                    waits = list(si.on_wait)
                    for w in waits[:-1]:
                        ev = mybir.InstEventSemaphore(
                            name=f"I-wsplit-{n}",
                            engine=inst.engine,
                            ins=[], outs=[],
                            sync_info=mybir.SyncInfo(on_wait=[w], on_update=[]),
                        )
                        n += 1
                        il.insert(i, ev)
                        i += 1
                    inst.sync_info = mybir.SyncInfo(on_wait=[waits[-1]],
                                                    on_update=list(si.on_update or []))
                i += 1
    return n


def _build_nc():
    import concourse.bass as bass
    import concourse.mybir as mybir
    from concourse.tile import TileContext

    f32 = mybir.dt.float32
    f32r = mybir.dt.float32r
    AF = mybir.ActivationFunctionType
    OP = mybir.AluOpType

    nc = bass.Bass()
    dp = lambda name, shape, dt_=f32, isOutput=False: nc.declare_dram_parameter(name, shape, dt_, isOutput)
    xin_d = dp("xin", [D, T])
    wq_d = dp("wq", [D + 1, _QKTOT])   # aligned cols; row 36 = bias (q pre-scaled)
    wk_d = dp("wk", [D + 1, _QKTOT])
    wv_d = dp("wv", [D + 1, _VTOT])    # [1|V]-interleaved; row36: 1.0 at ones-col, bv at V cols
    wo_d = dp("wo", [128, _GGTOT * D])
    ex_d = dp("ex", [4, _NEXK * 128])
    lnA_d = dp("lnA", [1, 2 * NSH * D])      # 36*g per (e,ln)
    lnB_d = dp("lnB", [2, 2 * NSH * D])  # rows {-g, b}
    bo_d = dp("bo", [D, NSH])
    w1_d = dp("w1", [D, NSH * DFF])
    b1_d = dp("b1", [72, 2 * NSH])           # col 2e+fold
    w2_d = dp("w2", [73, NSH * 2 * D])  # row 72: b2 on fold0, 0 on fold1
    out_d = dp("zout", [D, T], f32, True)

    with TileContext(nc) as tc:
        with tc.tile_pool(name="const", bufs=1) as cpool, \
             tc.tile_pool(name="state", bufs=1) as spool, \
             tc.tile_pool(name="work", bufs=2) as wpool, \
             tc.tile_pool(name="attn", bufs=1) as apool, \
             tc.tile_pool(name="ps1", bufs=2, space="PSUM") as p1, \
             tc.tile_pool(name="ps2", bufs=2, space="PSUM") as p2, \
             tc.tile_pool(name="ps3", bufs=2, space="PSUM") as p3:

            def load(dram, shape):
                t = cpool.tile(shape, dram.dtype, tag=dram.name, name=dram.name)
                nc.sync.dma_start(out=t[:], in_=dram[:])
                return t

            wq = load(wq_d, [D + 1, _QKTOT])
            wk = load(wk_d, [D + 1, _QKTOT])
            wv = load(wv_d, [D + 1, _VTOT])
            wo = load(wo_d, [128, _GGTOT * D])
            ex = load(ex_d, [4, _NEXK * 128])
            lnA = load(lnA_d, [1, 2 * NSH * D])
            lnB = load(lnB_d, [2, 2 * NSH * D])
            bo = load(bo_d, [D, NSH])
            w1 = load(w1_d, [D, NSH * DFF])
            b1 = load(b1_d, [72, 2 * NSH])
            w2 = load(w2_d, [73, NSH * 2 * D])

            epsb = cpool.tile([1, 1], f32, tag="epsb")
            nc.vector.memset(epsb[:], float(D * D * LN_EPS))
            # stats mask columns: col0 = z rows (0:36), col1 = x^2 rows (64:100)
            ones2 = cpool.tile([100, 2], f32, tag="ones2")
            nc.vector.memset(ones2[:], 0.0)
            nc.vector.memset(ones2[0:D, 0:1], 1.0)
            nc.vector.memset(ones2[64:100, 1:2], 1.0)

            # residual rows 0:36, zero pad 36:64, x^2 scratch rows 64:100
            zz = spool.tile([100, T], f32, tag="zz")
            nc.vector.memset(zz[32:64, :], 0.0)
            nc.sync.dma_start(out=zz[0:D, :], in_=xin_d[:])
            y37a = spool.tile([D + 1, T], f32, tag="y37a")
            y37f = spool.tile([D + 1, T], f32, tag="y37f")
            nc.vector.memset(y37a[:], 1.0)
            nc.vector.memset(y37f[:], 1.0)
            # row0 = rstd'*s1 (per LN), row1 = const ones
            st2a = spool.tile([2, L], f32, tag="st2a")
            st2f = spool.tile([2, L], f32, tag="st2f")
            nc.vector.memset(st2a[:], 1.0)
            nc.vector.memset(st2f[:], 1.0)
            ff = spool.tile([73, 2 * L], f32, tag="ff")
            nc.vector.memset(ff[:], 1.0)

            def layernorm(eln, toff, ydst, st2):
                """ydst[0:36, toff:toff+600] = LN(zz[0:36, toff:..]) with params lnA/lnB[eln]."""
                # x^2 on gpsimd
                nc.gpsimd.tensor_tensor(out=zz[64:100, toff:toff + L],
                                        in0=zz[0:D, toff:toff + L],
                                        in1=zz[0:D, toff:toff + L], op=OP.mult)
                s1b = wpool.tile([1, L], f32, tag="s1b", name="s1b", bufs=1)
                vsb = wpool.tile([1, L], f32, tag="vsb", name="vsb", bufs=1)
                qsb = wpool.tile([1, L], f32, tag="qsb", name="qsb", bufs=1)
                for c in range(2):
                    cs = slice(c * CH, (c + 1) * CH)
                    zsl = zz[:, toff + c * CH: toff + (c + 1) * CH]
                    ps1 = p1.tile([128, 512], f32, tag="ps", name="lns1")
                    ps2 = p1.tile([128, 512], f32, tag="ps", name="lns2")
                    nc.tensor.matmul(out=ps1[0:1, 0:CH], lhsT=ones2[:, 0:1], rhs=zsl)
                    nc.tensor.matmul(out=ps2[0:1, 0:CH], lhsT=ones2[:, 1:2], rhs=zsl)
                    nc.vector.tensor_copy(out=s1b[0:1, cs], in_=ps1[0:1, 0:CH])
                    nc.vector.tensor_tensor(out=qsb[0:1, cs], in0=s1b[0:1, cs],
                                            in1=s1b[0:1, cs], op=OP.mult)
                    nc.vector.scalar_tensor_tensor(out=vsb[0:1, cs], in0=ps2[0:1, 0:CH],
                                                   scalar=float(D), in1=qsb[0:1, cs],
                                                   op0=OP.mult, op1=OP.subtract)
                # rstd' = 1/sqrt(t + eps'): table sqrt + exact DVE reciprocal
                lvs = wpool.tile([1, L], f32, tag="lvs", name="lvs", bufs=1)
                nc.scalar.activation(out=lvs[:], in_=vsb[:], func=AF.Sqrt,
                                     bias=epsb[0:1, 0:1])
                rp = wpool.tile([1, L], f32, tag="rp", name="rp", bufs=1)
                nc.vector.reciprocal(out=rp[:], in_=lvs[:])
                nc.vector.tensor_tensor(out=st2[0:1, :], in0=rp[0:1, :],
                                        in1=s1b[0:1, :], op=OP.mult)
                for c in range(2):
                    pa_ = p1.tile([128, 512], f32, tag="ps", name="lnA")
                    pb_ = p1.tile([128, 512], f32, tag="ps", name="lnB")
                    nc.tensor.matmul(out=pa_[0:D, 0:CH],
                                     lhsT=lnA[0:1, eln * D:(eln + 1) * D],
                                     rhs=rp[0:1, c * CH:(c + 1) * CH],
                                     tile_position=(0, 0))
                    nc.tensor.matmul(out=pb_[0:D, 0:CH],
                                     lhsT=lnB[0:2, eln * D:(eln + 1) * D],
                                     rhs=st2[0:2, c * CH:(c + 1) * CH],
                                     tile_position=(0, 0))
                    ysl = ydst[0:D, toff + c * CH: toff + (c + 1) * CH]
                    nc.vector.tensor_tensor(out=ysl, in0=zz[0:D, toff + c * CH: toff + (c + 1) * CH],
                                            in1=pa_[0:D, 0:CH], op=OP.mult)
                    nc.vector.tensor_tensor(out=ysl, in0=ysl, in1=pb_[0:D, 0:CH], op=OP.add)

            for s, j, h in SH:
                e = ESH[(s, j)]
                dk, groups = _group_layout(h)
                krows = 36 if dk > 32 else 32
                toff = j * L
                vw = D + h
                vwe = vw + (vw % 2)
                voff = _VOFF[(s, j)]
                layernorm(2 * e, toff, y37a, st2a)

                # V^T + bias + ones via y37a as stationary
                vti = wpool.tile([MW, 5 * VSTR], f32, tag="vti", name="vti")
                for c in range(5):
                    pv = p1.tile([128, 512], f32, tag="ps", name="vtp")
                    nc.tensor.matmul(out=pv[0:MW, 0:vwe],
                                     lhsT=y37a[:, toff + c * MW: toff + (c + 1) * MW],
                                     rhs=wv[:, voff:voff + vwe],
                                     tile_position=(0, 0))
                    nc.vector.tensor_copy(out=vti[:, c * VSTR:c * VSTR + vw],
                                          in_=pv[0:MW, 0:vw])

                col = _QKOFF[(s, j)]
                for gl, (rows, strips) in enumerate(groups):
                    nh = len(strips)
                    gh = rows + 1 if h == 1 else rows
                    gg = _GGOFF[(s, j)] + gl
                    # Q/K projection (bias folded in row 36)
                    sq = p2.tile([128, 2, 512], f32, tag="sc", name="sq")
                    sk = p2.tile([128, 2, 512], f32, tag="sc", name="sk")
                    for c in range(2):
                        nc.tensor.matmul(out=sq[0:rows, c, 0:CH],
                                         lhsT=wq[:, col:col + rows],
                                         rhs=y37a[:, toff + c * CH: toff + (c + 1) * CH])
                        nc.tensor.matmul(out=sk[0:rows, c, 0:CH],
                                         lhsT=wk[:, col:col + rows],
                                         rhs=y37a[:, toff + c * CH: toff + (c + 1) * CH])
                    col += rows
                    qg = wpool.tile([128, L], f32, tag="qal", name="qg")
                    kg = wpool.tile([128, L], f32, tag="kal", name="kg")
                    nc.vector.tensor_copy(out=qg[0:rows, :], in_=sq[0:rows, :, 0:CH])
                    nc.vector.tensor_copy(out=kg[0:rows, :], in_=sk[0:rows, :, 0:CH])
                    # per-head c-pipeline: scores(c+1) overlaps exp/AV(c)
                    onorm = apool.tile([128, L], f32, tag="onorm", name="onorm",
                                       bufs=2)
                    zg = wpool.tile([4, L], f32, tag="zg", name="zg", bufs=1)
                    for hx, (strip, i) in enumerate(strips):
                        obase = strip if dk <= 31 else 0
                        poa_th = [p3.tile([128, 512], f32, tag="poa", name="poa")
                                  for _ in range(TH)]
                        for c in range(5):
                            sc_ = p2.tile([128, 2, 512], f32, tag="sc", name="sc")
                            for th in range(TH):
                                nc.tensor.matmul(
                                    out=sc_[0:MW, th, 0:CH],
                                    lhsT=kg[strip:strip + krows, c * MW:(c + 1) * MW],
                                    rhs=qg[strip:strip + krows, th * CH:(th + 1) * CH],
                                    tile_position=(strip, 0))
                            et = apool.tile([MW, 2 * CH], f32, tag=f"e{strip}",
                                            name=f"e{strip}", bufs=2)
                            nc.scalar.activation(out=et[:],
                                                 in_=sc_[0:MW, :, 0:CH], func=AF.Exp)
                            for th in range(TH):
                                nc.tensor.matmul(
                                    out=poa_th[th][0:dk + 1, 0:CH],
                                    lhsT=vti[:, c * VSTR + i * (dk + 1):
                                               c * VSTR + (i + 1) * (dk + 1)],
                                    rhs=et[:, th * CH:(th + 1) * CH],
                                    start=(c == 0), stop=(c == 4),
                                    tile_position=(0, 0))
                        for th in range(TH):
                            osl = onorm[obase:obase + dk + 1,
                                        th * CH:(th + 1) * CH]
                            if (hx + th) % 2 == 0:
                                nc.vector.tensor_copy(out=osl,
                                                      in_=poa_th[th][0:dk + 1, 0:CH])
                            else:
                                nc.scalar.activation(out=osl,
                                                     in_=poa_th[th][0:dk + 1, 0:CH],
                                                     func=AF.Copy)
                    for th in range(TH):
                        if dk <= 31:
                            nc.sync.dma_start(
                                out=zg[0:nh, th * CH:(th + 1) * CH],
                                in_=onorm[0:32 * nh:32, th * CH:(th + 1) * CH])
                        else:
                            nc.sync.dma_start(
                                out=zg[0:1, th * CH:(th + 1) * CH],
                                in_=onorm[0:1, th * CH:(th + 1) * CH])
                    rz = wpool.tile([4, L], f32, tag="rz", name="rz", bufs=1)
                    nc.vector.reciprocal(out=rz[0:nh, :], in_=zg[0:nh, :])
                    for th in range(TH):
                        pd = p1.tile([128, 512], f32, tag="ps", name="pd")
                        ek = _EXKEY[(dk, nh)]
                        nc.tensor.matmul(out=pd[0:gh, 0:CH],
                                         lhsT=ex[0:nh, ek * 128: ek * 128 + gh],
                                         rhs=rz[0:nh, th * CH:(th + 1) * CH],
                                         tile_position=(0, 0))
                        osl = onorm[0:gh, th * CH:(th + 1) * CH]
                        nc.vector.tensor_tensor(out=osl, in0=osl, in1=pd[0:gh, 0:CH],
                                                op=OP.mult)
                    for c in range(2):
                        po = p1.tile([128, 512], f32, tag="ps", name="po")
                        nc.tensor.matmul(out=po[0:D, 0:CH],
                                         lhsT=wo[0:gh, gg * D:(gg + 1) * D],
                                         rhs=onorm[0:gh, c * CH:(c + 1) * CH],
                                         tile_position=(0, 0))
                        zsl = zz[0:D, toff + c * CH: toff + (c + 1) * CH]
                        nc.vector.tensor_tensor(out=zsl, in0=zsl, in1=po[0:D, 0:CH],
                                                op=OP.add)
                zsl = zz[0:D, toff:toff + L]
                nc.vector.tensor_scalar(out=zsl, in0=zsl, scalar1=bo[:, e:e + 1],
                                        scalar2=None, op0=OP.add)

                # FFN
                layernorm(2 * e + 1, toff, y37f, st2f)
                for fold in range(2):
                    sf = p2.tile([128, 2, 512], f32, tag="sc", name="sf")
                    for c in range(2):
                        nc.tensor.matmul(
                            out=sf[0:72, c, 0:CH],
                            lhsT=w1[:, e * DFF + fold * 72: e * DFF + (fold + 1) * 72],
                            rhs=y37f[0:D, toff + c * CH: toff + (c + 1) * CH])
                    nc.scalar.activation(out=ff[0:72, fold * L:(fold + 1) * L],
                                         in_=sf[0:72, :, 0:CH], func=AF.Gelu_apprx_tanh,
                                         bias=b1[:, 2 * e + fold:2 * e + fold + 1])
                for c in range(2):
                    so = p1.tile([128, 512], f32, tag="ps", name="so")
                    for fold in range(2):
                        nc.tensor.matmul(
                            out=so[0:D, 0:CH],
                            lhsT=w2[:, (2 * e + fold) * D:(2 * e + fold + 1) * D],
                            rhs=ff[:, fold * L + c * CH: fold * L + (c + 1) * CH],
                            start=(fold == 0), stop=(fold == 1))
                    zsl = zz[0:D, toff + c * CH: toff + (c + 1) * CH]
                    nc.vector.tensor_tensor(out=zsl, in0=zsl, in1=so[0:D, 0:CH],
                                            op=OP.add)

            # SE gating per half, write out
            gated = spool.tile([D, T], f32, tag="gated")
            gsc = wpool.tile([D, 4], f32, tag="gsc")
            for j in range(2):
                toff = j * L
                nc.vector.tensor_reduce(out=gsc[:, j:j + 1], in_=zz[0:D, toff:toff + L],
                                        axis=mybir.AxisListType.X, op=OP.add)
                nc.scalar.activation(out=gsc[:, 2 + j:3 + j], in_=gsc[:, j:j + 1],
                                     func=AF.Sigmoid, scale=1.0 / L)
                nc.vector.tensor_scalar(out=gated[:, toff:toff + L],
                                        in0=zz[0:D, toff:toff + L],
                                        scalar1=gsc[:, 2 + j:3 + j],
                                        scalar2=None, op0=OP.mult)
            nc.sync.dma_start(out=out_d[:], in_=gated[:])
    return nc


def _build_nc2():
    """Launch 2: conv1d(144->36,k=3,pad=1) + BN + ReLU on a [144, 302] window."""
    import concourse.bass as bass
    import concourse.mybir as mybir
    from concourse.tile import TileContext

    f32 = mybir.dt.float32
    f32r = mybir.dt.float32r
    AF = mybir.ActivationFunctionType
    XW = 2 * CONV_W
    WW = 2 * 3 * D

    nc = bass.Bass()
    xfw_d = nc.declare_dram_parameter("xfw", [72, XW + WW], f32, False)
    sb_d = nc.declare_dram_parameter("sb2", [D, 2], f32, False)
    out_d = nc.declare_dram_parameter("yout", [D, TW], f32, True)

    with TileContext(nc) as tc:
        with tc.tile_pool(name="sb", bufs=1) as sb, \
             tc.tile_pool(name="ps", bufs=2, space="PSUM") as ps:
            xfw = sb.tile([72, XW + WW], f32, tag="xfw")
            nc.sync.dma_start(out=xfw[:], in_=xfw_d[:])
            sb2 = sb.tile([D, 2], f32, tag="sb2")
            nc.sync.dma_start(out=sb2[:], in_=sb_d[:])

            pso = ps.tile([D, TW], f32, tag="pso")
            first = True
            for k in range(3):
                for half in range(2):
                    nc.tensor.matmul(
                        out=pso[:],
                        lhsT=xfw[:, XW + half * 3 * D + k * D:
                                 XW + half * 3 * D + (k + 1) * D],
                        rhs=xfw[:, half * CONV_W + k: half * CONV_W + k + TW],
                        start=first, stop=(k == 2 and half == 1))
                    first = False
            yo = sb.tile([D, TW], f32, tag="yo")
            nc.scalar.activation(out=yo[:], in_=pso[:], func=AF.Relu,
                                 scale=sb2[:, 0:1], bias=sb2[:, 1:2])
            nc.sync.dma_start(out=out_d[:], in_=yo[:])
    return nc


_CACHE = {}
LAST_RESULTS = []


def _pack_core_weights(grp, Wq, bq, Wk, bk, Wv, bv, Wo, bo,
                       ln1_g, ln1_b, ln2_g, ln2_b, W1, b1, W2, b2):
    wq_p = np.zeros((D + 1, _QKTOT), np.float32)
    wk_p = np.zeros((D + 1, _QKTOT), np.float32)
    wv_p = np.zeros((D + 1, _VTOT), np.float32)
    wo_p = np.zeros((128, _GGTOT * D), np.float32)
    ex_p = np.zeros((4, _NEXK * 128), np.float32)
    lnA_p = np.zeros((1, 2 * NSH * D), np.float32)
    lnB_p = np.zeros((2, 2 * NSH * D), np.float32)
    bo_p = np.zeros((D, NSH), np.float32)
    w1_p = np.zeros((D, NSH * DFF), np.float32)
    b1_p = np.zeros((72, 2 * NSH), np.float32)
    w2_p = np.zeros((73, NSH * 2 * D), np.float32)

    for s, j, h in SH:
        e = ESH[(s, j)]
        dk, groups = _group_layout(h)
        li = _chain_layer(grp, s, j)
        # ones-cols of interleaved V are always 1.0 (Z row); exal mask always set
        voff = _VOFF[(s, j)]
        for i in range(h):
            wv_p[D, voff + i * (dk + 1)] = 1.0
        for gl, (rows, strips) in enumerate(groups):
            ek = _EXKEY[(dk, len(strips))]
            for jj, (strip, i) in enumerate(strips):
                base = 1 if h == 1 else strip + 1
                ex_p[jj, ek * 128 + base: ek * 128 + base + dk] = 1.0
        # LN params (identity: g=1, b=0)
        for ln in range(2):
            eln = 2 * e + ln
            if li is not None:
                g = (ln1_g if ln == 0 else ln2_g)[li]
                b = (ln1_b if ln == 0 else ln2_b)[li]
            else:
                g = np.ones(D, np.float32)
                b = np.zeros(D, np.float32)
            lnA_p[0, eln * D:(eln + 1) * D] = D * g
            lnB_p[0, eln * D:(eln + 1) * D] = -g
            lnB_p[1, eln * D:(eln + 1) * D] = b
        if li is None:
            continue
        sc = 1.0 / np.sqrt(dk)
        col = _QKOFF[(s, j)]
        for rows, strips in groups:
            for strip, i in strips:
                wq_p[0:D, col + strip: col + strip + dk] = Wq[li][:, i * dk:(i + 1) * dk] * sc
                wq_p[D, col + strip: col + strip + dk] = bq[li][i * dk:(i + 1) * dk] * sc
                wk_p[0:D, col + strip: col + strip + dk] = Wk[li][:, i * dk:(i + 1) * dk]
                wk_p[D, col + strip: col + strip + dk] = bk[li][i * dk:(i + 1) * dk]
            col += rows
        for i in range(h):
            c0 = voff + i * (dk + 1) + 1
            wv_p[0:D, c0:c0 + dk] = Wv[li][:, i * dk:(i + 1) * dk]
            wv_p[D, c0:c0 + dk] = bv[li][i * dk:(i + 1) * dk]
        for gl, (rows, strips) in enumerate(groups):
            gg = _GGOFF[(s, j)] + gl
            for strip, i in strips:
                base = 1 if h == 1 else strip + 1
                wo_p[base:base + dk, gg * D:(gg + 1) * D] = Wo[li][i * dk:(i + 1) * dk, :]
        bo_p[:, e] = bo[li]
        w1_p[:, e * DFF:(e + 1) * DFF] = W1[li]
        b1_p[:, 2 * e] = b1[li][:72]
        b1_p[:, 2 * e + 1] = b1[li][72:]
        w2_p[0:72, 2 * e * D:(2 * e + 1) * D] = W2[li][:72]
        w2_p[0:72, (2 * e + 1) * D:(2 * e + 2) * D] = W2[li][72:]
        w2_p[72, 2 * e * D:(2 * e + 1) * D] = b2[li]
    return dict(wq=wq_p, wk=wk_p, wv=wv_p, wo=wo_p, ex=ex_p, lnA=lnA_p,
                lnB=lnB_p, bo=bo_p, w1=w1_p, b1=b1_p, w2=w2_p)


def kernel(x, ln1_g, ln1_b, Wq, bq, Wk, bk, Wv, bv, Wo, bo,
           ln2_g, ln2_b, W1, b1, W2, b2,
           conv_w, conv_b, bn_g, bn_b, bn_mean, bn_var):
    from concourse.bass_utils import run_bass_kernel_spmd

    args = [np.asarray(a, np.float32) for a in
            (x, ln1_g, ln1_b, Wq, bq, Wk, bk, Wv, bv, Wo, bo,
             ln2_g, ln2_b, W1, b1, W2, b2)]
    (x, ln1_g, ln1_b, Wq, bq, Wk, bk, Wv, bv, Wo, bo,
     ln2_g, ln2_b, W1, b1, W2, b2) = args
    conv_w = np.asarray(conv_w, np.float32)
    conv_b = np.asarray(conv_b, np.float32)
    bn_g = np.asarray(bn_g, np.float32); bn_b = np.asarray(bn_b, np.float32)
    bn_mean = np.asarray(bn_mean, np.float32); bn_var = np.asarray(bn_var, np.float32)

    if "nc1" not in _CACHE:
        _CACHE["nc1"] = _build_nc()
        _split_multi_waits(_CACHE["nc1"])
        _CACHE["nc2"] = _build_nc2()
        _split_multi_waits(_CACHE["nc2"])
    nc1, nc2 = _CACHE["nc1"], _CACHE["nc2"]

    pe = _sin_pe()
    packed = {}
    in_maps = []
    for c in range(8):
        b, grp = c % 4, c // 4
        if grp not in packed:
            packed[grp] = _pack_core_weights(grp, Wq, bq, Wk, bk, Wv, bv, Wo, bo,
                                             ln1_g, ln1_b, ln2_g, ln2_b, W1, b1, W2, b2)
        xin = np.tile(x[b] + pe, (1, 2)).astype(np.float32)
        in_maps.append(dict(xin=xin, **packed[grp]))
    LAST_RESULTS.clear()
    r1 = run_bass_kernel_spmd(nc1, in_maps, list(range(8)))
    LAST_RESULTS.append(r1)
    res1 = r1.results

    # zout halves -> branches: core c (b=c%4, grp=c//4): half j -> branch 2*grp+j
    xf = np.zeros((B, DFF, L), np.float32)
    for c in range(8):
        b, grp = c % 4, c // 4
        zo = res1[c]["zout"]
        xf[b, (2 * grp) * D:(2 * grp + 1) * D] = zo[:, :L]
        xf[b, (2 * grp + 1) * D:(2 * grp + 2) * D] = zo[:, L:]

    scale = bn_g / np.sqrt(bn_var + BN_EPS)
    bias = bn_b + scale * (conv_b - bn_mean)
    wc = np.zeros((DFF, 3 * D), np.float32)
    for k in range(3):
        wc[:, k * D:(k + 1) * D] = conv_w[:, :, k].T
    wc2 = wc.reshape(2, 72, 3 * D).transpose(1, 0, 2).reshape(72, 2 * 3 * D).copy()
    in_maps2 = []
    for c in range(8):
        b, half = c // 2, c % 2
        win = np.zeros((DFF, CONV_W), np.float32)
        lo = half * TW - 1
        s0 = max(lo, 0)
        s1 = min(lo + CONV_W, L)
        win[:, s0 - lo: s1 - lo] = xf[b][:, s0:s1]
        win2 = win.reshape(2, 72, CONV_W).transpose(1, 0, 2).reshape(72, 2 * CONV_W)
        xfw = np.concatenate([win2, wc2], axis=1).astype(np.float32)
        sb2 = np.stack([scale, bias], axis=1).astype(np.float32)
        in_maps2.append(dict(xfw=xfw.copy(), sb2=sb2.copy()))
    r2 = run_bass_kernel_spmd(nc2, in_maps2, list(range(8)))
    LAST_RESULTS.append(r2)
    res2 = r2.results

    out = np.zeros((B, D, L), np.float32)
    for c in range(8):
        b, half = c // 2, c % 2
        out[b][:, half * TW:(half + 1) * TW] = res2[c]["yout"]
    return out
